# revision 61
# baseline (speedup 1.0000x reference)
"""VQ codebook encoding (soft-assignment aggregation) on 8 Trainium2 NeuronCores.

Reference computation (per batch b, with Xf = X[b] reshaped to [N, D]):
    dist[n,k] = ||x_n||^2 - 2<x_n, c_k> + ||c_k||^2
    A = softmax_k(scale_k * dist[n,k])
    E[k,d] = sum_n A[n,k] * Xf[n,d] - (sum_n A[n,k]) * C[k,d]

Sharding: data-parallel over B (8 batches -> 8 cores), no collectives.

Per-core dataflow (X[b] arrives d-major as [D=512, N=16384]):
  - For each tile of 128 n-values: load the four [128d x 128n] slices of X as
    PE weights once each; from the same weights issue (a) a matmul against a
    128x128 identity -> transposed tile Xf[n,d] in PSUM (needed because the
    output aggregation contracts over n, which must sit on partitions), and
    (b) a matmul against the pre-scaled codebook (-2*scale_k*C^T) -> the
    cross-term of the scaled distance, accumulated over the 4 d-chunks.
  - ScalarE copies Xf PSUM->SBUF (stream operand of the second matmul) and a
    Square-activation with accum_out produces ||x_n||^2 per partition.
  - VectorE assembles logits scale_k*(x2 - 2xc) and adds the scale_k*||c_k||^2
    bias; ScalarE exponentiates with accum_out producing the softmax
    denominator for free; reciprocal + tensor_scalar normalize.
  - PE accumulates E[k,d] (A as weights, Xf as stream) and S[k]=sum_n A[n,k]
    into persistent PSUM banks across all 128 n-tiles; the epilogue computes
    E - S*C and DMAs out [32, 512].
"""

import numpy as np

import concourse.bass as bass
import concourse.tile as tile
from concourse import bacc, mybir
from concourse.bass_utils import run_bass_kernel_spmd

F32 = mybir.dt.float32
BF16 = mybir.dt.bfloat16
AF = mybir.ActivationFunctionType
ALU = mybir.AluOpType

B, D, K, N = 8, 512, 32, 16384
P = 128                 # partitions
DC = D // P             # 4 d-chunks
NT = N // P             # 128 n-tiles per core
SG_N = 2048             # n-values per DMA super-group (1 MiB per d-chunk slice)
NSG = N // SG_N         # 8 super-groups
TPG = SG_N // P         # 16 n-tiles per super-group


def _build_bass():
    nc = bacc.Bacc(None, target_bir_lowering=False)

    x_d = nc.declare_dram_parameter("x", [D, N], F32, isOutput=False)
    ctm2s_d = nc.declare_dram_parameter("ctm2s", [D, K], BF16, isOutput=False)
    scaleb_d = nc.declare_dram_parameter("scaleb", [P, K], F32, isOutput=False)
    bb_d = nc.declare_dram_parameter("bb", [P, K], F32, isOutput=False)
    ident_d = nc.declare_dram_parameter("ident", [P, P], BF16, isOutput=False)
    ones_d = nc.declare_dram_parameter("onescol", [P, 1], BF16, isOutput=False)
    cs_d = nc.declare_dram_parameter("cs", [K, D], F32, isOutput=False)
    e_d = nc.declare_dram_parameter("e", [K, D], F32, isOutput=True)

    with tile.TileContext(nc) as tc:
        with (
            tc.tile_pool(name="consts", bufs=1) as cpool,
            tc.tile_pool(name="xin", bufs=2 * DC) as xin_pool,
            tc.tile_pool(name="xf_sb", bufs=4) as xf_pool,
            tc.tile_pool(name="smalls", bufs=8) as sm_pool,
            tc.tile_pool(name="scratch", bufs=1) as scr_pool,
            tc.tile_pool(name="xf_ps", bufs=4, space="PSUM") as xfps_pool,
            tc.tile_pool(name="sl_ps", bufs=2, space="PSUM") as slps_pool,
            tc.tile_pool(name="acc_ps", bufs=1, space="PSUM") as accps_pool,
        ):
            # ---- constants to SBUF ----
            ctm2s = cpool.tile([P, DC, K], BF16)  # chunk c at [:, c, :]
            nc.sync.dma_start(
                ctm2s[:], ctm2s_d.rearrange("(c p) k -> p c k", p=P)
            )
            scaleb = cpool.tile([P, K], F32)
            nc.sync.dma_start(scaleb[:], scaleb_d[:])
            bb = cpool.tile([P, K], F32)
            nc.sync.dma_start(bb[:], bb_d[:])
            ident = cpool.tile([P, P], BF16)
            nc.sync.dma_start(ident[:], ident_d[:])
            onescol = cpool.tile([P, 1], BF16)
            nc.sync.dma_start(onescol[:], ones_d[:])
            cs = cpool.tile([K, D], F32)
            nc.sync.dma_start(cs[:], cs_d[:])

            e_ps = accps_pool.tile([K, D], F32)
            s_ps = accps_pool.tile([K, 1], F32)
            sq_scr = scr_pool.tile([P, DC * P], BF16)  # dead store for Square

            for sg in range(NSG):
                xin16 = []
                for c in range(DC):
                    # X arrives f32 in HBM; SWDGE casts to bf16 in-flight.
                    # bf16 is plenty end-to-end here (verified vs f64 model):
                    # the logit noise it adds washes out of E.
                    t16 = xin_pool.tile([P, SG_N], BF16, tag="xin16")
                    nc.gpsimd.dma_start(
                        t16[:], x_d[c * P:(c + 1) * P, sg * SG_N:(sg + 1) * SG_N]
                    )
                    xin16.append(t16)

                for ti in range(TPG):
                    nt = sg * TPG + ti
                    xf_ps = xfps_pool.tile([P, DC * P], BF16)
                    sl_ps = slps_pool.tile([P, K], F32)
                    for c in range(DC):
                        # transpose-mode matmul (bf16 in -> bf16 PSUM)
                        nc.tensor.transpose(
                            xf_ps[:, c * P:(c + 1) * P],
                            xin16[c][:, ti * P:(ti + 1) * P], ident[:],
                        )
                        # cross-term: -2*scale_k*<x_n, c_k>, accumulated (bf16)
                        nc.tensor.matmul(
                            sl_ps[:],
                            xin16[c][:, ti * P:(ti + 1) * P], ctm2s[:, c, :],
                            start=(c == 0), stop=(c == DC - 1),
                        )

                    # Xf PSUM -> SBUF bf16 early (parallel with the softmax
                    # chain below; E-matmul stream operand)
                    xf_sb = xf_pool.tile([P, DC * P], BF16)
                    nc.vector.tensor_copy(xf_sb[:], xf_ps[:])

                    # x2[n] = sum_d Xf[n,d]^2 via Square + accumulate
                    x2 = sm_pool.tile([P, 1], F32, tag="x2")
                    nc.scalar.activation(
                        sq_scr[:], xf_ps[:], AF.Square, accum_out=x2[:]
                    )

                    # logits: scale_k * x2[n] + (-2 scale_k xc)
                    sl_sb = sm_pool.tile([P, K], F32, tag="sl")
                    nc.vector.scalar_tensor_tensor(
                        sl_sb[:], scaleb[:], x2[:], sl_ps[:],
                        op0=ALU.mult, op1=ALU.add,
                    )

                    # + scale_k*c2_k bias -> full scaled distance (<= 0)
                    sl2 = sm_pool.tile([P, K], F32, tag="sl2")
                    nc.vector.tensor_add(sl2[:], sl_sb[:], bb[:])

                    # Q = exp(logits) in bf16, unnormalized (normalization is
                    # folded into the Xf copy below)
                    q = sm_pool.tile([P, K], BF16, tag="q")
                    nc.scalar.activation(q[:], sl2[:], AF.Exp)

                    denom = sm_pool.tile([P, 1], F32, tag="den")
                    nc.vector.reduce_sum(denom[:], q[:], axis=mybir.AxisListType.X)
                    rcol = sm_pool.tile([P, 1], F32, tag="rc")
                    nc.vector.reciprocal(rcol[:], denom[:])

                    # A = Q / denom (bf16, 2x DVE mode)
                    a = sm_pool.tile([P, K], BF16, tag="a")
                    nc.vector.tensor_scalar_mul(a[:], q[:], rcol[:])

                    # E[k,d] += A.T @ Xf ; S[k] += A.T @ 1
                    nc.tensor.matmul(
                        e_ps[:], a[:], xf_sb[:],
                        start=(nt == 0), stop=(nt == NT - 1),
                        skip_group_check=True,
                    )
                    nc.tensor.matmul(
                        s_ps[:], a[:], onescol[:],
                        start=(nt == 0), stop=(nt == NT - 1),
                        skip_group_check=True,
                    )

            # epilogue: E = e_ps - S*C
            s_neg = sm_pool.tile([K, 1], F32, tag="sn")
            nc.scalar.activation(s_neg[:], s_ps[:], AF.Copy, scale=-1.0)
            e_sb = xf_pool.tile([K, D], F32, tag="eout")
            nc.vector.scalar_tensor_tensor(
                e_sb[:], cs[:], s_neg[:], e_ps[:],
                op0=ALU.mult, op1=ALU.add,
            )
            nc.sync.dma_start(e_d[:], e_sb[:])

    nc.compile()
    return nc


_CACHED = {}


def _get_nc():
    if "nc" not in _CACHED:
        _CACHED["nc"] = _build_bass()
    return _CACHED["nc"]


def kernel(X, codewords, scale, _trace=False):
    X = np.asarray(X, dtype=np.float32)
    codewords = np.asarray(codewords, dtype=np.float32)
    scale = np.asarray(scale, dtype=np.float32)

    Xr = np.ascontiguousarray(X.reshape(B, D, N))

    c2 = (codewords.astype(np.float64) ** 2).sum(axis=1)
    import ml_dtypes
    ctm2s = np.ascontiguousarray(
        (-2.0 * scale[None, :] * codewords.T).astype(ml_dtypes.bfloat16)
    )
    scaleb = np.broadcast_to(scale[None, :], (P, K)).copy()
    bb = np.broadcast_to(
        (scale.astype(np.float64) * c2).astype(np.float32)[None, :], (P, K)
    ).copy()
    ident = np.eye(P, dtype=ml_dtypes.bfloat16)
    onescol = np.ones((P, 1), dtype=ml_dtypes.bfloat16)
    cs = np.ascontiguousarray(codewords)

    consts = dict(
        ctm2s=ctm2s, scaleb=scaleb, bb=bb,
        ident=ident, onescol=onescol, cs=cs,
    )
    in_maps = [dict(x=np.ascontiguousarray(Xr[b]), **consts) for b in range(B)]

    nc = _get_nc()
    res = run_bass_kernel_spmd(nc, in_maps, list(range(B)), trace=_trace)
    out = np.stack([res.results[b]["e"] for b in range(B)]).astype(np.float32)
    if _trace:
        kernel.last_results = res
    return out


# revision 63
# speedup vs baseline: 1.3282x; 1.3282x over previous
"""VQ codebook encoding (soft-assignment aggregation) on 8 Trainium2 NeuronCores.

Reference computation (per batch b, with Xf = X[b] reshaped to [N, D]):
    dist[n,k] = ||x_n||^2 - 2<x_n, c_k> + ||c_k||^2
    A = softmax_k(scale_k * dist[n,k])
    E[k,d] = sum_n A[n,k] * Xf[n,d] - (sum_n A[n,k]) * C[k,d]

Sharding: data-parallel over B (8 batches -> 8 cores), no collectives.

Per-core dataflow (X[b] arrives d-major as [D=512, N=16384]):
  - For each tile of 128 n-values: load the four [128d x 128n] slices of X as
    PE weights once each; from the same weights issue (a) a matmul against a
    128x128 identity -> transposed tile Xf[n,d] in PSUM (needed because the
    output aggregation contracts over n, which must sit on partitions), and
    (b) a matmul against the pre-scaled codebook (-2*scale_k*C^T) -> the
    cross-term of the scaled distance, accumulated over the 4 d-chunks.
  - ScalarE copies Xf PSUM->SBUF (stream operand of the second matmul) and a
    Square-activation with accum_out produces ||x_n||^2 per partition.
  - VectorE assembles logits scale_k*(x2 - 2xc) and adds the scale_k*||c_k||^2
    bias; ScalarE exponentiates with accum_out producing the softmax
    denominator for free; reciprocal + tensor_scalar normalize.
  - PE accumulates E[k,d] (A as weights, Xf as stream) and S[k]=sum_n A[n,k]
    into persistent PSUM banks across all 128 n-tiles; the epilogue computes
    E - S*C and DMAs out [32, 512].
"""

import numpy as np

import concourse.bass as bass
import concourse.tile as tile
from concourse import bacc, mybir
from concourse.bass_utils import run_bass_kernel_spmd

F32 = mybir.dt.float32
BF16 = mybir.dt.bfloat16
AF = mybir.ActivationFunctionType
ALU = mybir.AluOpType

B, D, K, N = 8, 512, 32, 16384
P = 128                 # partitions
DC = D // P             # 4 d-chunks
NT = N // P             # 128 n-tiles per core
SG_N = 2048             # n-values per DMA super-group (1 MiB per d-chunk slice)
NSG = N // SG_N         # 8 super-groups
TPG = SG_N // P         # 16 n-tiles per super-group


def _build_bass():
    nc = bacc.Bacc(None, target_bir_lowering=False)

    x_d = nc.declare_dram_parameter("x", [D, N], F32, isOutput=False)
    ctm2s_d = nc.declare_dram_parameter("ctm2s", [D, K], BF16, isOutput=False)
    scaleb_d = nc.declare_dram_parameter("scaleb", [P, K], F32, isOutput=False)
    bb_d = nc.declare_dram_parameter("bb", [P, K], F32, isOutput=False)
    ident_d = nc.declare_dram_parameter("ident", [P, P], BF16, isOutput=False)
    ones_d = nc.declare_dram_parameter("onescol", [P, 1], BF16, isOutput=False)
    cs_d = nc.declare_dram_parameter("cs", [K, D], F32, isOutput=False)
    e_d = nc.declare_dram_parameter("e", [K, D], F32, isOutput=True)

    with tile.TileContext(nc) as tc:
        with (
            tc.tile_pool(name="consts", bufs=1) as cpool,
            tc.tile_pool(name="xin", bufs=2 * DC) as xin_pool,
            tc.tile_pool(name="xf_sb", bufs=4) as xf_pool,
            tc.tile_pool(name="smalls", bufs=8) as sm_pool,
            tc.tile_pool(name="scratch", bufs=1) as scr_pool,
            tc.tile_pool(name="xf_ps", bufs=4, space="PSUM") as xfps_pool,
            tc.tile_pool(name="sl_ps", bufs=2, space="PSUM") as slps_pool,
            tc.tile_pool(name="acc_ps", bufs=1, space="PSUM") as accps_pool,
        ):
            # ---- constants to SBUF ----
            ctm2s = cpool.tile([P, DC, K], BF16)  # chunk c at [:, c, :]
            nc.sync.dma_start(
                ctm2s[:], ctm2s_d.rearrange("(c p) k -> p c k", p=P)
            )
            scaleb = cpool.tile([P, K], F32)
            nc.sync.dma_start(scaleb[:], scaleb_d[:])
            bb = cpool.tile([P, K], F32)
            nc.sync.dma_start(bb[:], bb_d[:])
            ident = cpool.tile([P, P], BF16)
            nc.sync.dma_start(ident[:], ident_d[:])
            onescol = cpool.tile([P, 1], BF16)
            nc.sync.dma_start(onescol[:], ones_d[:])
            cs = cpool.tile([K, D], F32)
            nc.sync.dma_start(cs[:], cs_d[:])

            e_ps = accps_pool.tile([K, D], F32)
            s_ps = accps_pool.tile([K, 1], F32)
            sq_scr = scr_pool.tile([P, DC * P], BF16)  # dead store for Square

            for sg in range(NSG):
                xin16 = []
                for c in range(DC):
                    # X arrives f32 in HBM; SWDGE casts to bf16 in-flight.
                    # bf16 is plenty end-to-end here (verified vs f64 model):
                    # the logit noise it adds washes out of E.
                    t16 = xin_pool.tile([P, SG_N], BF16, tag="xin16")
                    nc.gpsimd.dma_start(
                        t16[:], x_d[c * P:(c + 1) * P, sg * SG_N:(sg + 1) * SG_N]
                    )
                    xin16.append(t16)

                for ti in range(TPG):
                    nt = sg * TPG + ti
                    xf_ps = xfps_pool.tile([P, DC * P], BF16)
                    sl_ps = slps_pool.tile([P, K], F32)
                    for c in range(DC):
                        # transpose-mode matmul (bf16 in -> bf16 PSUM)
                        nc.tensor.transpose(
                            xf_ps[:, c * P:(c + 1) * P],
                            xin16[c][:, ti * P:(ti + 1) * P], ident[:],
                        )
                        # cross-term: -2*scale_k*<x_n, c_k>, accumulated (bf16)
                        nc.tensor.matmul(
                            sl_ps[:],
                            xin16[c][:, ti * P:(ti + 1) * P], ctm2s[:, c, :],
                            start=(c == 0), stop=(c == DC - 1),
                        )

                    # x2[n] = sum_d Xf[n,d]^2 via Square + accumulate
                    x2 = sm_pool.tile([P, 1], F32, tag="x2")
                    nc.scalar.activation(
                        sq_scr[:], xf_ps[:], AF.Square, accum_out=x2[:]
                    )

                    # logits: scale_k * x2[n] + (-2 scale_k xc)
                    sl_sb = sm_pool.tile([P, K], F32, tag="sl")
                    nc.vector.scalar_tensor_tensor(
                        sl_sb[:], scaleb[:], x2[:], sl_ps[:],
                        op0=ALU.mult, op1=ALU.add,
                    )

                    # + scale_k*c2_k bias -> full scaled distance (<= 0)
                    sl2 = sm_pool.tile([P, K], F32, tag="sl2")
                    nc.vector.tensor_add(sl2[:], sl_sb[:], bb[:])

                    # Q = exp(logits) in bf16, unnormalized (normalization is
                    # folded into the Xf copy below)
                    q = sm_pool.tile([P, K], BF16, tag="q")
                    nc.scalar.activation(q[:], sl2[:], AF.Exp)

                    denom = sm_pool.tile([P, 1], F32, tag="den")
                    nc.vector.reduce_sum(denom[:], q[:], axis=mybir.AxisListType.X)
                    rcol = sm_pool.tile([P, 1], F32, tag="rc")
                    nc.vector.reciprocal(rcol[:], denom[:])
                    rcol16 = sm_pool.tile([P, 1], BF16, tag="rc16")
                    nc.vector.tensor_copy(rcol16[:], rcol[:])

                    # Xf PSUM -> SBUF as bf16, pre-scaled by 1/denom(n)
                    # (per-partition scale; bf16 src enables the DVE 2x mode)
                    xf_sb = xf_pool.tile([P, DC * P], BF16)
                    nc.vector.tensor_scalar_mul(xf_sb[:], xf_ps[:], rcol[:])

                    # E[k,d] += Q.T @ (Xf/denom) ; S[k] += Q.T @ (1/denom)
                    nc.tensor.matmul(
                        e_ps[:], q[:], xf_sb[:],
                        start=(nt == 0), stop=(nt == NT - 1),
                        skip_group_check=True,
                    )
                    nc.tensor.matmul(
                        s_ps[:], q[:], rcol16[:],
                        start=(nt == 0), stop=(nt == NT - 1),
                        skip_group_check=True,
                    )

            # epilogue: E = e_ps - S*C
            s_neg = sm_pool.tile([K, 1], F32, tag="sn")
            nc.scalar.activation(s_neg[:], s_ps[:], AF.Copy, scale=-1.0)
            e_sb = xf_pool.tile([K, D], F32, tag="eout")
            nc.vector.scalar_tensor_tensor(
                e_sb[:], cs[:], s_neg[:], e_ps[:],
                op0=ALU.mult, op1=ALU.add,
            )
            nc.sync.dma_start(e_d[:], e_sb[:])

    nc.compile()
    return nc


_CACHED = {}


def _get_nc():
    if "nc" not in _CACHED:
        _CACHED["nc"] = _build_bass()
    return _CACHED["nc"]


def kernel(X, codewords, scale, _trace=False):
    X = np.asarray(X, dtype=np.float32)
    codewords = np.asarray(codewords, dtype=np.float32)
    scale = np.asarray(scale, dtype=np.float32)

    Xr = np.ascontiguousarray(X.reshape(B, D, N))

    c2 = (codewords.astype(np.float64) ** 2).sum(axis=1)
    import ml_dtypes
    ctm2s = np.ascontiguousarray(
        (-2.0 * scale[None, :] * codewords.T).astype(ml_dtypes.bfloat16)
    )
    scaleb = np.broadcast_to(scale[None, :], (P, K)).copy()
    bb = np.broadcast_to(
        (scale.astype(np.float64) * c2).astype(np.float32)[None, :], (P, K)
    ).copy()
    ident = np.eye(P, dtype=ml_dtypes.bfloat16)
    onescol = np.ones((P, 1), dtype=ml_dtypes.bfloat16)
    cs = np.ascontiguousarray(codewords)

    consts = dict(
        ctm2s=ctm2s, scaleb=scaleb, bb=bb,
        ident=ident, onescol=onescol, cs=cs,
    )
    in_maps = [dict(x=np.ascontiguousarray(Xr[b]), **consts) for b in range(B)]

    nc = _get_nc()
    res = run_bass_kernel_spmd(nc, in_maps, list(range(B)), trace=_trace)
    out = np.stack([res.results[b]["e"] for b in range(B)]).astype(np.float32)
    if _trace:
        kernel.last_results = res
    return out


# revision 67
# speedup vs baseline: 1.3415x; 1.0100x over previous
"""VQ codebook encoding (soft-assignment aggregation) on 8 Trainium2 NeuronCores.

Reference computation (per batch b, with Xf = X[b] reshaped to [N, D]):
    dist[n,k] = ||x_n||^2 - 2<x_n, c_k> + ||c_k||^2
    A = softmax_k(scale_k * dist[n,k])
    E[k,d] = sum_n A[n,k] * Xf[n,d] - (sum_n A[n,k]) * C[k,d]

Sharding: data-parallel over B (8 batches -> 8 cores), no collectives.

Per-core dataflow (X[b] arrives d-major as [D=512, N=16384]):
  - For each tile of 128 n-values: load the four [128d x 128n] slices of X as
    PE weights once each; from the same weights issue (a) a matmul against a
    128x128 identity -> transposed tile Xf[n,d] in PSUM (needed because the
    output aggregation contracts over n, which must sit on partitions), and
    (b) a matmul against the pre-scaled codebook (-2*scale_k*C^T) -> the
    cross-term of the scaled distance, accumulated over the 4 d-chunks.
  - ScalarE copies Xf PSUM->SBUF (stream operand of the second matmul) and a
    Square-activation with accum_out produces ||x_n||^2 per partition.
  - VectorE assembles logits scale_k*(x2 - 2xc) and adds the scale_k*||c_k||^2
    bias; ScalarE exponentiates with accum_out producing the softmax
    denominator for free; reciprocal + tensor_scalar normalize.
  - PE accumulates E[k,d] (A as weights, Xf as stream) and S[k]=sum_n A[n,k]
    into persistent PSUM banks across all 128 n-tiles; the epilogue computes
    E - S*C and DMAs out [32, 512].
"""

import numpy as np

import concourse.bass as bass
import concourse.tile as tile
from concourse import bacc, mybir
from concourse.bass_utils import run_bass_kernel_spmd

F32 = mybir.dt.float32
BF16 = mybir.dt.bfloat16
AF = mybir.ActivationFunctionType
ALU = mybir.AluOpType

B, D, K, N = 8, 512, 32, 16384
P = 128                 # partitions
DC = D // P             # 4 d-chunks
NT = N // P             # 128 n-tiles per core
SG_N = 2048             # n-values per DMA super-group (1 MiB per d-chunk slice)
NSG = N // SG_N         # 8 super-groups
TPG = SG_N // P         # 16 n-tiles per super-group


def _build_bass():
    nc = bacc.Bacc(None, target_bir_lowering=False)

    x_d = nc.declare_dram_parameter("x", [D, N], F32, isOutput=False)
    ctm2s_d = nc.declare_dram_parameter("ctm2s", [D, K], BF16, isOutput=False)
    scaleb_d = nc.declare_dram_parameter("scaleb", [P, K], F32, isOutput=False)
    bb_d = nc.declare_dram_parameter("bb", [P, K], F32, isOutput=False)
    ident_d = nc.declare_dram_parameter("ident", [P, P], BF16, isOutput=False)
    cs_d = nc.declare_dram_parameter("cs", [K, D], F32, isOutput=False)
    e_d = nc.declare_dram_parameter("e", [K, D], F32, isOutput=True)

    with tile.TileContext(nc) as tc:
        with (
            tc.tile_pool(name="consts", bufs=1) as cpool,
            tc.tile_pool(name="xin", bufs=2 * DC) as xin_pool,
            tc.tile_pool(name="xf_sb", bufs=4) as xf_pool,
            tc.tile_pool(name="smalls", bufs=8) as sm_pool,
            tc.tile_pool(name="scratch", bufs=1) as scr_pool,
            tc.tile_pool(name="xf_ps", bufs=4, space="PSUM") as xfps_pool,
            tc.tile_pool(name="sl_ps", bufs=2, space="PSUM") as slps_pool,
            tc.tile_pool(name="acc_ps", bufs=1, space="PSUM") as accps_pool,
        ):
            # ---- constants to SBUF ----
            ctm2s = cpool.tile([P, DC, K], BF16)  # chunk c at [:, c, :]
            nc.sync.dma_start(
                ctm2s[:], ctm2s_d.rearrange("(c p) k -> p c k", p=P)
            )
            scaleb = cpool.tile([P, K], F32)
            nc.sync.dma_start(scaleb[:], scaleb_d[:])
            bb = cpool.tile([P, K], F32)
            nc.sync.dma_start(bb[:], bb_d[:])
            ident = cpool.tile([P, P], BF16)
            nc.sync.dma_start(ident[:], ident_d[:])
            cs = cpool.tile([K, D], F32)
            nc.sync.dma_start(cs[:], cs_d[:])

            e_ps = accps_pool.tile([K, D], F32)
            s_ps = accps_pool.tile([K, 1], F32)
            sq_scr = scr_pool.tile([P, DC * P], BF16)  # dead store for Square

            for sg in range(NSG):
                xin16 = []
                for c in range(DC):
                    # X arrives f32 in HBM; SWDGE casts to bf16 in-flight.
                    # bf16 is plenty end-to-end here (verified vs f64 model):
                    # the logit noise it adds washes out of E.
                    t16 = xin_pool.tile([P, SG_N], BF16, tag="xin16")
                    nc.gpsimd.dma_start(
                        t16[:], x_d[c * P:(c + 1) * P, sg * SG_N:(sg + 1) * SG_N]
                    )
                    xin16.append(t16)

                for ti in range(TPG):
                    nt = sg * TPG + ti
                    xf_ps = xfps_pool.tile([P, DC * P], BF16)
                    sl_ps = slps_pool.tile([P, K], F32)
                    for c in range(DC):
                        # transpose-mode matmul (bf16 in -> bf16 PSUM)
                        nc.tensor.transpose(
                            xf_ps[:, c * P:(c + 1) * P],
                            xin16[c][:, ti * P:(ti + 1) * P], ident[:],
                        )
                        # cross-term: -2*scale_k*<x_n, c_k>, accumulated (bf16)
                        nc.tensor.matmul(
                            sl_ps[:],
                            xin16[c][:, ti * P:(ti + 1) * P], ctm2s[:, c, :],
                            start=(c == 0), stop=(c == DC - 1),
                        )

                    # x2[n] = sum_d Xf[n,d]^2 via Square + accumulate
                    x2 = sm_pool.tile([P, 1], F32, tag="x2")
                    nc.scalar.activation(
                        sq_scr[:], xf_ps[:], AF.Square, accum_out=x2[:]
                    )

                    # logits: scale_k * x2[n] + (-2 scale_k xc)
                    sl_sb = sm_pool.tile([P, K], F32, tag="sl")
                    nc.vector.scalar_tensor_tensor(
                        sl_sb[:], scaleb[:], x2[:], sl_ps[:],
                        op0=ALU.mult, op1=ALU.add,
                    )

                    # + scale_k*c2_k bias -> full scaled distance (<= 0)
                    sl2 = sm_pool.tile([P, K], F32, tag="sl2")
                    nc.vector.tensor_add(sl2[:], sl_sb[:], bb[:])

                    # Q = exp(logits) in bf16, unnormalized (normalization is
                    # folded into the Xf copy below)
                    q = sm_pool.tile([P, K], BF16, tag="q")
                    nc.scalar.activation(q[:], sl2[:], AF.Exp)

                    denom = sm_pool.tile([P, 1], F32, tag="den")
                    nc.vector.reduce_sum(denom[:], q[:], axis=mybir.AxisListType.X)
                    rcol = sm_pool.tile([P, 1], F32, tag="rc")
                    nc.vector.reciprocal(rcol[:], denom[:])
                    rcol16 = sm_pool.tile([P, 1], BF16, tag="rc16")
                    nc.vector.tensor_copy(rcol16[:], rcol[:])

                    # Xf PSUM -> SBUF as bf16, pre-scaled by 1/denom(n)
                    # (per-partition scale; bf16 src enables the DVE 2x mode)
                    xf_sb = xf_pool.tile([P, DC * P], BF16)
                    nc.vector.tensor_scalar_mul(xf_sb[:], xf_ps[:], rcol[:])

                    # E[k,d] += Q.T @ (Xf/denom) ; S[k] += Q.T @ (1/denom)
                    nc.tensor.matmul(
                        e_ps[:], q[:], xf_sb[:],
                        start=(nt == 0), stop=(nt == NT - 1),
                        skip_group_check=True,
                    )
                    nc.tensor.matmul(
                        s_ps[:], q[:], rcol16[:],
                        start=(nt == 0), stop=(nt == NT - 1),
                        skip_group_check=True,
                    )

            # epilogue: E = e_ps - S*C
            s_neg = sm_pool.tile([K, 1], F32, tag="sn")
            nc.scalar.activation(s_neg[:], s_ps[:], AF.Copy, scale=-1.0)
            e_sb = xf_pool.tile([K, D], F32, tag="eout")
            nc.vector.scalar_tensor_tensor(
                e_sb[:], cs[:], s_neg[:], e_ps[:],
                op0=ALU.mult, op1=ALU.add,
            )
            nc.sync.dma_start(e_d[:], e_sb[:])

    nc.compile()
    return nc


_CACHED = {}


def _get_nc():
    if "nc" not in _CACHED:
        _CACHED["nc"] = _build_bass()
    return _CACHED["nc"]


def kernel(X, codewords, scale, _trace=False):
    X = np.asarray(X, dtype=np.float32)
    codewords = np.asarray(codewords, dtype=np.float32)
    scale = np.asarray(scale, dtype=np.float32)

    Xr = np.ascontiguousarray(X.reshape(B, D, N))

    c2 = (codewords.astype(np.float64) ** 2).sum(axis=1)
    import ml_dtypes
    ctm2s = np.ascontiguousarray(
        (-2.0 * scale[None, :] * codewords.T).astype(ml_dtypes.bfloat16)
    )
    scaleb = np.broadcast_to(scale[None, :], (P, K)).copy()
    bb = np.broadcast_to(
        (scale.astype(np.float64) * c2).astype(np.float32)[None, :], (P, K)
    ).copy()
    ident = np.eye(P, dtype=ml_dtypes.bfloat16)
    cs = np.ascontiguousarray(codewords)

    consts = dict(
        ctm2s=ctm2s, scaleb=scaleb, bb=bb,
        ident=ident, cs=cs,
    )
    in_maps = [dict(x=np.ascontiguousarray(Xr[b]), **consts) for b in range(B)]

    nc = _get_nc()
    res = run_bass_kernel_spmd(nc, in_maps, list(range(B)), trace=_trace)
    out = np.stack([res.results[b]["e"] for b in range(B)]).astype(np.float32)
    if _trace:
        kernel.last_results = res
    return out


# revision 68
# speedup vs baseline: 1.3468x; 1.0040x over previous
"""VQ codebook encoding (soft-assignment aggregation) on 8 Trainium2 NeuronCores.

Reference computation (per batch b, with Xf = X[b] reshaped to [N, D]):
    dist[n,k] = ||x_n||^2 - 2<x_n, c_k> + ||c_k||^2
    A = softmax_k(scale_k * dist[n,k])
    E[k,d] = sum_n A[n,k] * Xf[n,d] - (sum_n A[n,k]) * C[k,d]

Sharding: data-parallel over B (8 batches -> 8 cores), no collectives.

Per-core dataflow (X[b] arrives d-major as [D=512, N=16384]):
  - For each tile of 128 n-values: load the four [128d x 128n] slices of X as
    PE weights once each; from the same weights issue (a) a matmul against a
    128x128 identity -> transposed tile Xf[n,d] in PSUM (needed because the
    output aggregation contracts over n, which must sit on partitions), and
    (b) a matmul against the pre-scaled codebook (-2*scale_k*C^T) -> the
    cross-term of the scaled distance, accumulated over the 4 d-chunks.
  - ScalarE copies Xf PSUM->SBUF (stream operand of the second matmul) and a
    Square-activation with accum_out produces ||x_n||^2 per partition.
  - VectorE assembles logits scale_k*(x2 - 2xc) and adds the scale_k*||c_k||^2
    bias; ScalarE exponentiates with accum_out producing the softmax
    denominator for free; reciprocal + tensor_scalar normalize.
  - PE accumulates E[k,d] (A as weights, Xf as stream) and S[k]=sum_n A[n,k]
    into persistent PSUM banks across all 128 n-tiles; the epilogue computes
    E - S*C and DMAs out [32, 512].
"""

import numpy as np

import concourse.bass as bass
import concourse.tile as tile
from concourse import bacc, mybir
from concourse.bass_utils import run_bass_kernel_spmd

F32 = mybir.dt.float32
BF16 = mybir.dt.bfloat16
AF = mybir.ActivationFunctionType
ALU = mybir.AluOpType

B, D, K, N = 8, 512, 32, 16384
P = 128                 # partitions
DC = D // P             # 4 d-chunks
NT = N // P             # 128 n-tiles per core
SG_N = 2048             # n-values per DMA super-group (1 MiB per d-chunk slice)
NSG = N // SG_N         # 8 super-groups
TPG = SG_N // P         # 16 n-tiles per super-group


def _build_bass():
    nc = bacc.Bacc(None, target_bir_lowering=False)

    x_d = nc.declare_dram_parameter("x", [D, N], F32, isOutput=False)
    ctm2s_d = nc.declare_dram_parameter("ctm2s", [D, K], BF16, isOutput=False)
    scaleb_d = nc.declare_dram_parameter("scaleb", [P, K], F32, isOutput=False)
    bb_d = nc.declare_dram_parameter("bb", [P, K], F32, isOutput=False)
    ident_d = nc.declare_dram_parameter("ident", [P, P], BF16, isOutput=False)
    cs_d = nc.declare_dram_parameter("cs", [K, D], F32, isOutput=False)
    e_d = nc.declare_dram_parameter("e", [K, D], F32, isOutput=True)

    with tile.TileContext(nc) as tc:
        with (
            tc.tile_pool(name="consts", bufs=1) as cpool,
            tc.tile_pool(name="xin", bufs=3 * DC) as xin_pool,
            tc.tile_pool(name="xf_sb", bufs=6) as xf_pool,
            tc.tile_pool(name="smalls", bufs=12) as sm_pool,
            tc.tile_pool(name="scratch", bufs=1) as scr_pool,
            tc.tile_pool(name="xf_ps", bufs=4, space="PSUM") as xfps_pool,
            tc.tile_pool(name="sl_ps", bufs=2, space="PSUM") as slps_pool,
            tc.tile_pool(name="acc_ps", bufs=1, space="PSUM") as accps_pool,
        ):
            # ---- constants to SBUF ----
            ctm2s = cpool.tile([P, DC, K], BF16)  # chunk c at [:, c, :]
            nc.sync.dma_start(
                ctm2s[:], ctm2s_d.rearrange("(c p) k -> p c k", p=P)
            )
            scaleb = cpool.tile([P, K], F32)
            nc.sync.dma_start(scaleb[:], scaleb_d[:])
            bb = cpool.tile([P, K], F32)
            nc.sync.dma_start(bb[:], bb_d[:])
            ident = cpool.tile([P, P], BF16)
            nc.sync.dma_start(ident[:], ident_d[:])
            cs = cpool.tile([K, D], F32)
            nc.sync.dma_start(cs[:], cs_d[:])

            e_ps = accps_pool.tile([K, D], F32)
            s_ps = accps_pool.tile([K, 1], F32)
            sq_scr = scr_pool.tile([P, DC * P], BF16)  # dead store for Square

            for sg in range(NSG):
                xin16 = []
                for c in range(DC):
                    # X arrives f32 in HBM; SWDGE casts to bf16 in-flight.
                    # bf16 is plenty end-to-end here (verified vs f64 model):
                    # the logit noise it adds washes out of E.
                    t16 = xin_pool.tile([P, SG_N], BF16, tag="xin16")
                    nc.gpsimd.dma_start(
                        t16[:], x_d[c * P:(c + 1) * P, sg * SG_N:(sg + 1) * SG_N]
                    )
                    xin16.append(t16)

                for ti in range(TPG):
                    nt = sg * TPG + ti
                    xf_ps = xfps_pool.tile([P, DC * P], BF16)
                    sl_ps = slps_pool.tile([P, K], F32)
                    for c in range(DC):
                        # transpose-mode matmul (bf16 in -> bf16 PSUM)
                        nc.tensor.transpose(
                            xf_ps[:, c * P:(c + 1) * P],
                            xin16[c][:, ti * P:(ti + 1) * P], ident[:],
                        )
                        # cross-term: -2*scale_k*<x_n, c_k>, accumulated (bf16)
                        nc.tensor.matmul(
                            sl_ps[:],
                            xin16[c][:, ti * P:(ti + 1) * P], ctm2s[:, c, :],
                            start=(c == 0), stop=(c == DC - 1),
                        )

                    # x2[n] = sum_d Xf[n,d]^2 via Square + accumulate
                    x2 = sm_pool.tile([P, 1], F32, tag="x2")
                    nc.scalar.activation(
                        sq_scr[:], xf_ps[:], AF.Square, accum_out=x2[:]
                    )

                    # logits: scale_k * x2[n] + (-2 scale_k xc)
                    sl_sb = sm_pool.tile([P, K], F32, tag="sl")
                    nc.vector.scalar_tensor_tensor(
                        sl_sb[:], scaleb[:], x2[:], sl_ps[:],
                        op0=ALU.mult, op1=ALU.add,
                    )

                    # + scale_k*c2_k bias -> full scaled distance (<= 0)
                    sl2 = sm_pool.tile([P, K], F32, tag="sl2")
                    nc.vector.tensor_add(sl2[:], sl_sb[:], bb[:])

                    # Q = exp(logits) in bf16, unnormalized (normalization is
                    # folded into the Xf copy below)
                    q = sm_pool.tile([P, K], BF16, tag="q")
                    nc.scalar.activation(q[:], sl2[:], AF.Exp)

                    denom = sm_pool.tile([P, 1], F32, tag="den")
                    nc.vector.reduce_sum(denom[:], q[:], axis=mybir.AxisListType.X)
                    rcol = sm_pool.tile([P, 1], F32, tag="rc")
                    nc.vector.reciprocal(rcol[:], denom[:])
                    rcol16 = sm_pool.tile([P, 1], BF16, tag="rc16")
                    nc.vector.tensor_copy(rcol16[:], rcol[:])

                    # Xf PSUM -> SBUF as bf16, pre-scaled by 1/denom(n)
                    # (per-partition scale; bf16 src enables the DVE 2x mode)
                    xf_sb = xf_pool.tile([P, DC * P], BF16)
                    nc.vector.tensor_scalar_mul(xf_sb[:], xf_ps[:], rcol[:])

                    # E[k,d] += Q.T @ (Xf/denom) ; S[k] += Q.T @ (1/denom)
                    nc.tensor.matmul(
                        e_ps[:], q[:], xf_sb[:],
                        start=(nt == 0), stop=(nt == NT - 1),
                        skip_group_check=True,
                    )
                    nc.tensor.matmul(
                        s_ps[:], q[:], rcol16[:],
                        start=(nt == 0), stop=(nt == NT - 1),
                        skip_group_check=True,
                    )

            # epilogue: E = e_ps - S*C
            s_neg = sm_pool.tile([K, 1], F32, tag="sn")
            nc.scalar.activation(s_neg[:], s_ps[:], AF.Copy, scale=-1.0)
            e_sb = xf_pool.tile([K, D], F32, tag="eout")
            nc.vector.scalar_tensor_tensor(
                e_sb[:], cs[:], s_neg[:], e_ps[:],
                op0=ALU.mult, op1=ALU.add,
            )
            nc.sync.dma_start(e_d[:], e_sb[:])

    nc.compile()
    return nc


_CACHED = {}


def _get_nc():
    if "nc" not in _CACHED:
        _CACHED["nc"] = _build_bass()
    return _CACHED["nc"]


def kernel(X, codewords, scale, _trace=False):
    X = np.asarray(X, dtype=np.float32)
    codewords = np.asarray(codewords, dtype=np.float32)
    scale = np.asarray(scale, dtype=np.float32)

    Xr = np.ascontiguousarray(X.reshape(B, D, N))

    c2 = (codewords.astype(np.float64) ** 2).sum(axis=1)
    import ml_dtypes
    ctm2s = np.ascontiguousarray(
        (-2.0 * scale[None, :] * codewords.T).astype(ml_dtypes.bfloat16)
    )
    scaleb = np.broadcast_to(scale[None, :], (P, K)).copy()
    bb = np.broadcast_to(
        (scale.astype(np.float64) * c2).astype(np.float32)[None, :], (P, K)
    ).copy()
    ident = np.eye(P, dtype=ml_dtypes.bfloat16)
    cs = np.ascontiguousarray(codewords)

    consts = dict(
        ctm2s=ctm2s, scaleb=scaleb, bb=bb,
        ident=ident, cs=cs,
    )
    in_maps = [dict(x=np.ascontiguousarray(Xr[b]), **consts) for b in range(B)]

    nc = _get_nc()
    res = run_bass_kernel_spmd(nc, in_maps, list(range(B)), trace=_trace)
    out = np.stack([res.results[b]["e"] for b in range(B)]).astype(np.float32)
    if _trace:
        kernel.last_results = res
    return out


# revision 69
# speedup vs baseline: 1.3552x; 1.0062x over previous
"""VQ codebook encoding (soft-assignment aggregation) on 8 Trainium2 NeuronCores.

Reference computation (per batch b, with Xf = X[b] reshaped to [N, D]):
    dist[n,k] = ||x_n||^2 - 2<x_n, c_k> + ||c_k||^2
    A = softmax_k(scale_k * dist[n,k])
    E[k,d] = sum_n A[n,k] * Xf[n,d] - (sum_n A[n,k]) * C[k,d]

Sharding: data-parallel over B (8 batches -> 8 cores), no collectives.

Per-core dataflow (X[b] arrives d-major as [D=512, N=16384] f32 in HBM):
  - SWDGE DMA loads X and casts f32 -> bf16 in flight (HBM reads the full
    33.5 MB once; only 16.7 MB lands in SBUF). bf16 end-to-end was validated
    against an f64 model: the logit noise it adds washes out of E
    (~2e-3 scale-relative max error).
  - Per 128-n tile: four bf16 transpose-mode matmuls produce Xf[n,d] in a
    bf16 PSUM tile (the output aggregation contracts over n, which must sit
    on partitions), and four matmuls against the pre-scaled codebook
    (-2*scale_k*C^T) accumulate the distance cross-term.
  - ScalarE Square-activation with accum_out produces ||x_n||^2; VectorE
    assembles logits scale_k*(x2 - 2xc) + scale_k*||c_k||^2; ScalarE
    exponentiates to unnormalized bf16 Q; VectorE computes the softmax
    denominator + reciprocal, and normalization is folded into the
    PSUM->SBUF copy of Xf (per-partition scale, bf16 2x DVE mode).
  - PE accumulates E[k,d] (Q as weights, Xf/denom as stream) and
    S[k] = sum_n Q/denom into persistent PSUM banks across all 128 n-tiles;
    the epilogue computes E - S*C and DMAs out [32, 512] f32.
"""

import numpy as np

import concourse.bass as bass
import concourse.tile as tile
from concourse import bacc, mybir
from concourse.bass_utils import run_bass_kernel_spmd

F32 = mybir.dt.float32
BF16 = mybir.dt.bfloat16
AF = mybir.ActivationFunctionType
ALU = mybir.AluOpType

B, D, K, N = 8, 512, 32, 16384
P = 128                 # partitions
DC = D // P             # 4 d-chunks
NT = N // P             # 128 n-tiles per core
SG_N = 2048             # n-values per DMA super-group (1 MiB per d-chunk slice)
NSG = N // SG_N         # 8 super-groups
TPG = SG_N // P         # 16 n-tiles per super-group


def _build_bass():
    nc = bacc.Bacc(None, target_bir_lowering=False)

    x_d = nc.declare_dram_parameter("x", [D, N], F32, isOutput=False)
    ctm2s_d = nc.declare_dram_parameter("ctm2s", [D, K], BF16, isOutput=False)
    scaleb_d = nc.declare_dram_parameter("scaleb", [P, K], F32, isOutput=False)
    bb_d = nc.declare_dram_parameter("bb", [P, K], F32, isOutput=False)
    ident_d = nc.declare_dram_parameter("ident", [P, P], BF16, isOutput=False)
    cs_d = nc.declare_dram_parameter("cs", [K, D], F32, isOutput=False)
    e_d = nc.declare_dram_parameter("e", [K, D], F32, isOutput=True)

    with tile.TileContext(nc) as tc:
        with (
            tc.tile_pool(name="consts", bufs=1) as cpool,
            tc.tile_pool(name="xin", bufs=3 * DC) as xin_pool,
            tc.tile_pool(name="xf_sb", bufs=6) as xf_pool,
            tc.tile_pool(name="smalls", bufs=12) as sm_pool,
            tc.tile_pool(name="scratch", bufs=1) as scr_pool,
            tc.tile_pool(name="xf_ps", bufs=4, space="PSUM") as xfps_pool,
            tc.tile_pool(name="sl_ps", bufs=2, space="PSUM") as slps_pool,
            tc.tile_pool(name="acc_ps", bufs=1, space="PSUM") as accps_pool,
        ):
            # ---- constants to SBUF ----
            ctm2s = cpool.tile([P, DC, K], BF16)  # chunk c at [:, c, :]
            nc.sync.dma_start(
                ctm2s[:], ctm2s_d.rearrange("(c p) k -> p c k", p=P)
            )
            scaleb = cpool.tile([P, K], F32)
            nc.sync.dma_start(scaleb[:], scaleb_d[:])
            bb = cpool.tile([P, K], F32)
            nc.sync.dma_start(bb[:], bb_d[:])
            ident = cpool.tile([P, P], BF16)
            nc.sync.dma_start(ident[:], ident_d[:])
            cs = cpool.tile([K, D], F32)
            nc.sync.dma_start(cs[:], cs_d[:])

            e_ps = accps_pool.tile([K, D], F32)
            s_ps = accps_pool.tile([K, 1], F32)
            sq_scr = scr_pool.tile([P, DC * P], BF16)  # dead store for Square

            for sg in range(NSG):
                xin16 = []
                for c in range(DC):
                    # X arrives f32 in HBM; SWDGE casts to bf16 in-flight.
                    # bf16 is plenty end-to-end here (verified vs f64 model):
                    # the logit noise it adds washes out of E.
                    t16 = xin_pool.tile([P, SG_N], BF16, tag="xin16")
                    nc.gpsimd.dma_start(
                        t16[:], x_d[c * P:(c + 1) * P, sg * SG_N:(sg + 1) * SG_N]
                    )
                    xin16.append(t16)

                for ti in range(TPG):
                    nt = sg * TPG + ti
                    xf_ps = xfps_pool.tile([P, DC * P], BF16)
                    sl_ps = slps_pool.tile([P, K], F32)
                    for c in range(DC):
                        # transpose-mode matmul (bf16 in -> bf16 PSUM)
                        nc.tensor.transpose(
                            xf_ps[:, c * P:(c + 1) * P],
                            xin16[c][:, ti * P:(ti + 1) * P], ident[:],
                        )
                        # cross-term: -2*scale_k*<x_n, c_k>, accumulated (bf16)
                        nc.tensor.matmul(
                            sl_ps[:],
                            xin16[c][:, ti * P:(ti + 1) * P], ctm2s[:, c, :],
                            start=(c == 0), stop=(c == DC - 1),
                        )

                    # x2[n] = sum_d Xf[n,d]^2 via Square + accumulate
                    x2 = sm_pool.tile([P, 1], F32, tag="x2")
                    nc.scalar.activation(
                        sq_scr[:], xf_ps[:], AF.Square, accum_out=x2[:]
                    )

                    # logits: scale_k * x2[n] + (-2 scale_k xc)
                    sl_sb = sm_pool.tile([P, K], F32, tag="sl")
                    nc.vector.scalar_tensor_tensor(
                        sl_sb[:], scaleb[:], x2[:], sl_ps[:],
                        op0=ALU.mult, op1=ALU.add,
                    )

                    # + scale_k*c2_k bias -> full scaled distance (<= 0)
                    sl2 = sm_pool.tile([P, K], F32, tag="sl2")
                    nc.vector.tensor_add(sl2[:], sl_sb[:], bb[:])

                    # Q = exp(logits) in bf16, unnormalized (normalization is
                    # folded into the Xf copy below)
                    q = sm_pool.tile([P, K], BF16, tag="q")
                    nc.scalar.activation(q[:], sl2[:], AF.Exp)

                    denom = sm_pool.tile([P, 1], F32, tag="den")
                    nc.vector.reduce_sum(denom[:], q[:], axis=mybir.AxisListType.X)
                    rcol = sm_pool.tile([P, 1], F32, tag="rc")
                    nc.vector.reciprocal(rcol[:], denom[:])
                    rcol16 = sm_pool.tile([P, 1], BF16, tag="rc16")
                    nc.vector.tensor_copy(rcol16[:], rcol[:])

                    # Xf PSUM -> SBUF as bf16, pre-scaled by 1/denom(n)
                    # (per-partition scale; bf16 src enables the DVE 2x mode)
                    xf_sb = xf_pool.tile([P, DC * P], BF16)
                    nc.vector.tensor_scalar_mul(xf_sb[:], xf_ps[:], rcol[:])

                    # E[k,d] += Q.T @ (Xf/denom) ; S[k] += Q.T @ (1/denom)
                    nc.tensor.matmul(
                        e_ps[:], q[:], xf_sb[:],
                        start=(nt == 0), stop=(nt == NT - 1),
                        skip_group_check=True,
                    )
                    nc.tensor.matmul(
                        s_ps[:], q[:], rcol16[:],
                        start=(nt == 0), stop=(nt == NT - 1),
                        skip_group_check=True,
                    )

            # epilogue: E = e_ps - S*C
            s_neg = sm_pool.tile([K, 1], F32, tag="sn")
            nc.scalar.activation(s_neg[:], s_ps[:], AF.Copy, scale=-1.0)
            e_sb = xf_pool.tile([K, D], F32, tag="eout")
            nc.vector.scalar_tensor_tensor(
                e_sb[:], cs[:], s_neg[:], e_ps[:],
                op0=ALU.mult, op1=ALU.add,
            )
            nc.sync.dma_start(e_d[:], e_sb[:])

    nc.compile()
    return nc


_CACHED = {}


def _get_nc():
    if "nc" not in _CACHED:
        _CACHED["nc"] = _build_bass()
    return _CACHED["nc"]


def kernel(X, codewords, scale, _trace=False):
    X = np.asarray(X, dtype=np.float32)
    codewords = np.asarray(codewords, dtype=np.float32)
    scale = np.asarray(scale, dtype=np.float32)

    Xr = np.ascontiguousarray(X.reshape(B, D, N))

    c2 = (codewords.astype(np.float64) ** 2).sum(axis=1)
    import ml_dtypes
    ctm2s = np.ascontiguousarray(
        (-2.0 * scale[None, :] * codewords.T).astype(ml_dtypes.bfloat16)
    )
    scaleb = np.broadcast_to(scale[None, :], (P, K)).copy()
    bb = np.broadcast_to(
        (scale.astype(np.float64) * c2).astype(np.float32)[None, :], (P, K)
    ).copy()
    ident = np.eye(P, dtype=ml_dtypes.bfloat16)
    cs = np.ascontiguousarray(codewords)

    consts = dict(
        ctm2s=ctm2s, scaleb=scaleb, bb=bb,
        ident=ident, cs=cs,
    )
    in_maps = [dict(x=np.ascontiguousarray(Xr[b]), **consts) for b in range(B)]

    nc = _get_nc()
    res = run_bass_kernel_spmd(nc, in_maps, list(range(B)), trace=_trace)
    out = np.stack([res.results[b]["e"] for b in range(B)]).astype(np.float32)
    if _trace:
        kernel.last_results = res
    return out
